# revision 17
# baseline (speedup 1.0000x reference)
"""Multi-head attention (16 heads, d_model=1024, S=2048) on 8 Trainium2 cores.

Tensor-parallel over heads: each core owns 2 heads (its 128-feature slice).
The host sums the 8 row-parallel partials and adds the folded bias.

v2 design (vs the v1 baseline's 126us):
  - Algebraic folds kill both projections:
      scores = (Q Wq.T + bq)(K Wk.T).T  ==  (Q G + g) K.T
        with G = Wq.T Wk / 8,  g = Wk.T bq / 8  (host-precomputed, bf16)
      o @ Wo.T  ==  (A V_raw) @ U.T  with U = Wo_slice Wv (host, bf16)
    so the kernel runs ONE projection (qm = G.T @ QT + g) and uses RAW
    K chunks as the scores stationaries and RAW V chunks (host-quantized
    e4m3, with ones columns for the softmax denominators) as the attn@V
    stationaries.  bv folds into bo on the host; bk drops (softmax shift
    invariance).
  - mm2 (scores): per-head K=64 row-tiled matmul pairs run CONCURRENTLY
    in disjoint PE row-groups (measured 119ns per N=512 MM vs 220 solo).
  - mm3 (attn@V): fp8e4 DoubleRow packs chunk-pairs (2x contraction per
    instruction, measured 218ns per N=512 MM = 2 chunks of work).
  - exp: e4m3 softmax weights; ACT does true Exp for 9/16 chunks per
    segment, DVE does a one-op Schraudolph (e4m3 bits = s*A + B as uint8,
    HW rounds) for 7/16.  Only ACT/DVE can read PSUM (gpsimd cannot),
    which makes PSUM evacuation the wall this design is balanced around.
  - Softmax normalize: denominators ride the ones columns into row 64 of
    the fp32 accumulators; reciprocal via the DMA spread/broadcast trick;
    the oT multiplies land on gpsimd/DVE.
  - Final linear: U-stationary, 8 N=512 quarters per 512-col segment,
    woven one segment behind the attention loop so output DMA streams.
"""

import sys

for _p in ("/opt/trn_rl_repo",):
    if _p not in sys.path:
        sys.path.insert(0, _p)

from contextlib import ExitStack

import ml_dtypes
import numpy as np

import concourse.bass as bass
import concourse.tile as tile
from concourse import bacc, mybir
from concourse.bass import ts
from concourse.bass_utils import run_bass_kernel_spmd

S = 2048          # sequence length
DK = 1024         # d_model
H = 16            # heads
DH = 64           # head dim
NCORES = 8
CW = 128          # per-core feature slice (2 heads x 64)
NCH = 16          # 128-row key chunks
NSEG = 4          # 512-col query segments
VPW = 320         # bytes per chunk-pair in VP: 2 k-tiles x (64 v + 1 + 15 pad)

F32 = mybir.dt.float32
BF16 = mybir.dt.bfloat16
E4 = mybir.dt.float8e4
U8 = mybir.dt.uint8
EXP = mybir.ActivationFunctionType.Exp
DR = mybir.MatmulPerfMode.DoubleRow

# e4m3-bits Schraudolph: bits8(exp(s)) ~= round(s * A + B) (HW rounds on
# the fp32->u8 convert; B tuned for min max-rel-err over the normal range)
SCH_A = float(np.float32(8.0 * np.log2(np.e)))
SCH_B = 55.62
# chunks per segment whose exp runs on ACT (rest on DVE Schraudolph)
ACT_SET = frozenset((0, 2, 4, 6, 8, 9, 11, 13, 15))

N_WARMUP_MM = 5

_CACHE = {}


def _build_nc():
    nc = bacc.Bacc(
        "TRN2", target_bir_lowering=False, debug=False, enable_asserts=False
    )

    QTd = nc.dram_tensor("QTd", [CW, S], BF16, kind="ExternalInput")
    KTd = nc.dram_tensor("KTd", [CW, S], BF16, kind="ExternalInput")
    # raw V chunk-pair stationaries (e4m3) with ones cols at m=64 per head
    VPd = nc.dram_tensor("VPd", [CW, (NCH // 2) * VPW], E4, kind="ExternalInput")
    Gd = nc.dram_tensor("Gd", [CW, 64], BF16, kind="ExternalInput")
    Ud = nc.dram_tensor("Ud", [CW, DK], BF16, kind="ExternalInput")
    Gq = nc.dram_tensor("Gq", [CW, 1], F32, kind="ExternalInput")
    PT = nc.dram_tensor("PT", [DK, S], BF16, kind="ExternalOutput")
    # last segment exported unnormalized, per head; host divides by DN
    PTB = nc.dram_tensor("PTB", [DK, 1024], BF16, kind="ExternalOutput")
    DN = nc.dram_tensor("DN", [1, 1024], F32, kind="ExternalOutput")

    with tile.TileContext(nc) as tc:
        with ExitStack() as ctx:
            pers = ctx.enter_context(tc.tile_pool(name="pers", bufs=1))
            expool = ctx.enter_context(tc.tile_pool(name="expool", bufs=3))
            stg = ctx.enter_context(tc.tile_pool(name="stg", bufs=4))
            nrm = ctx.enter_context(tc.tile_pool(name="nrm", bufs=2))
            scp = ctx.enter_context(tc.tile_pool(name="scp", bufs=3, space="PSUM"))
            accp = ctx.enter_context(tc.tile_pool(name="accp", bufs=1, space="PSUM"))
            dscr = ctx.enter_context(tc.tile_pool(name="dscr", bufs=2, space="DRAM"))

            # ---- t=0: ACT exp table preload + PE warmup fodder ----
            warm = pers.tile([128, 512], BF16, tag="warm")
            nc.vector.memset(warm[:], 0.0)
            wex = pers.tile([128, 1], F32, tag="wex")
            nc.scalar.activation(wex[:], warm[:, 0:1], EXP)

            # ---- input loads on the two HW-DGE queues ----
            KT = pers.tile([128, S], BF16, tag="KT")
            QT = pers.tile([128, S], BF16, tag="QT")
            VP = pers.tile([128, (NCH // 2) * VPW], E4, tag="VP")
            G_sb = pers.tile([128, 64], BF16, tag="G")
            U_sb = pers.tile([128, DK], BF16, tag="U")
            gq_sb = pers.tile([128, 1], F32, tag="gq")

            nc.scalar.dma_start(G_sb[:], Gd.ap())
            nc.scalar.dma_start(gq_sb[:], Gq.ap())
            nc.sync.dma_start(KT[:, ts(0, 1024)], KTd.ap()[:, ts(0, 1024)])
            nc.scalar.dma_start(QT[:, ts(0, 1024)], QTd.ap()[:, ts(0, 1024)])
            nc.sync.dma_start(KT[:, ts(1, 1024)], KTd.ap()[:, ts(1, 1024)])
            nc.scalar.dma_start(QT[:, ts(1, 1024)], QTd.ap()[:, ts(1, 1024)])
            nc.sync.dma_start(VP[:], VPd.ap())
            nc.scalar.dma_start(U_sb[:], Ud.ap())

            # ---- PE warmup stream (HAM ramp; no readers) ----
            for _ in range(N_WARMUP_MM):
                pw = scp.tile([128, 1024], F32, tag="sc")
                nc.tensor.matmul(pw[:, 0:512], warm[:, 0:128], warm[:])

            # ---- qm projection: qmT = G.T @ QT (+ g), row-tiled pairs ----
            qm = pers.tile([128, S], BF16, tag="qm")
            for sl in range(4):
                pq = scp.tile([128, 1024], F32, tag="sc")
                nc.tensor.matmul(pq[0:64, 0:512], G_sb[0:64, :], QT[0:64, ts(sl, 512)])
                nc.tensor.matmul(
                    pq[64:128, 0:512], G_sb[64:128, :], QT[64:128, ts(sl, 512)]
                )
                nc.vector.tensor_scalar_add(
                    qm[:, ts(sl, 512)], pq[:, 0:512], gq_sb[:]
                )

            # ---- attention + woven final linear ----
            oT = pers.tile([128, S], BF16, tag="oT")

            def vp_view(p, h):
                return VP[:, p * VPW : (p + 1) * VPW].rearrange(
                    "q (i c) -> q i c", i=2
                )[:, :, h * 80 : h * 80 + 65]

            def fl_unit(s, u):
                """Final-linear super-unit: row-blocks 2u, 2u+1 -> one
                [128,1024] psum tile -> one staged copy -> one DMA."""

                def emit():
                    p = scp.tile([128, 1024], F32, tag="sc")
                    nc.tensor.matmul(
                        p[:, 0:512], U_sb[:, ts(2 * u, 128)], oT[:, ts(s, 512)]
                    )
                    nc.tensor.matmul(
                        p[:, 512:1024], U_sb[:, ts(2 * u + 1, 128)], oT[:, ts(s, 512)]
                    )
                    st = stg.tile([128, 1024], BF16, tag="st")
                    if u % 2 == 0:
                        nc.scalar.copy(st[:], p[:])
                    else:
                        nc.vector.tensor_copy(st[:], p[:])
                    nc.sync.dma_start(
                        PT.ap()[
                            2 * u * 128 : (2 * u + 2) * 128,
                            s * 512 : (s + 1) * 512,
                        ].rearrange("(i p) f -> p i f", i=2),
                        st[:].rearrange("p (i f) -> p i f", i=2),
                    )

                return emit

            # per-seg normalize state handed to the next segment
            ACT_EXP = frozenset((0, 2, 4, 6, 8, 10, 12, 14, 15))

            def segment(s, extras=(), normalize=True):
                """extras: final-linear closures popped at even chunks
                >= 8 (the previous segment's oT is ready by ~chunk 5)."""
                extras = list(extras)
                acc0 = accp.tile([65, 512], F32, tag="acc0")
                acc1 = accp.tile([65, 512], F32, tag="acc1")
                acc = [acc0, acc1]
                ep = None
                mm3q = []

                def mm3_maker(p, ep_):
                    def emit():
                        for h in (0, 1):
                            nc.tensor.matmul(
                                acc[h][:],
                                vp_view(p, h),
                                ep_[:, :, h * 512 : (h + 1) * 512],
                                start=(p == 0),
                                stop=(p == 7),
                                perf_mode=DR,
                            )

                    return emit

                for j in range(NCH):
                    sc = scp.tile([128, 1024], F32, tag="sc")
                    nc.tensor.matmul(
                        sc[:, 0:512], KT[0:64, ts(j, 128)], qm[0:64, ts(s, 512)]
                    )
                    nc.tensor.matmul(
                        sc[:, 512:1024], KT[64:128, ts(j, 128)], qm[64:128, ts(s, 512)]
                    )
                    if j % 2 == 1 and len(mm3q) == 2:
                        mm3q.pop(0)()
                    if j % 2 == 0:
                        ep = expool.tile([128, 2, 1024], E4, tag="ep")
                    esl = ep[:, j % 2, :]
                    if j in ACT_EXP:
                        nc.scalar.activation(esl, sc[:], EXP)
                    else:
                        nc.vector.tensor_scalar(
                            esl.bitcast(U8), sc[:], SCH_A, SCH_B,
                            op0=mybir.AluOpType.mult, op1=mybir.AluOpType.add,
                        )
                    if j % 2 == 1:
                        mm3q.append(mm3_maker(j // 2, ep))
                    if j >= 8 and j % 2 == 0 and extras:
                        extras.pop(0)()

                # boundary: flush lagged mm3 pairs, interleaved with leftover
                # extras so the PE has independent work while exp(15) drains
                if extras:
                    extras.pop(0)()
                mm3q.pop(0)()
                while extras:
                    extras.pop(0)()
                mm3q.pop(0)()
                if not normalize:
                    return acc0, acc1

                # normalize, short-latency: 1-lane reciprocals straight from
                # the PSUM den rows, one broadcast DMA, gpsimd multiplies.
                rr = nrm.tile([1, 1024], F32, tag="rr")
                nc.vector.reciprocal(rr[0:1, 0:512], acc0[64:65, :])
                nc.vector.reciprocal(rr[0:1, 512:1024], acc1[64:65, :])
                oc0 = nrm.tile([65, 512], F32, tag="oc0")
                nc.scalar.copy(oc0[:], acc0[:])
                oc1 = nrm.tile([65, 512], F32, tag="oc1")
                nc.scalar.copy(oc1[:], acc1[:])
                rrd = dscr.tile([1, 1024], F32, tag="rrd")
                nc.sync.dma_start(rrd[:], rr[:])
                rb0 = nrm.tile([64, 512], F32, tag="rb0")
                nc.sync.dma_start(rb0[:], rrd[0:1, 0:512].to_broadcast((64, 512)))
                rb1 = nrm.tile([64, 512], F32, tag="rb1")
                nc.sync.dma_start(rb1[:], rrd[0:1, 512:1024].to_broadcast((64, 512)))
                nc.gpsimd.tensor_mul(oT[0:64, ts(s, 512)], oc0[0:64, :], rb0[:])
                nc.gpsimd.tensor_mul(oT[64:128, ts(s, 512)], oc1[0:64, :], rb1[:])
                return None

            segment(0)
            segment(1, extras=[fl_unit(0, u) for u in range(4)])
            segment(2, extras=[fl_unit(1, u) for u in range(4)])
            acc0, acc1 = segment(
                3, extras=[fl_unit(2, u) for u in range(4)], normalize=False
            )
            # tail: export seg 3 unnormalized per head (host divides by DN) --
            # no normalize chain gates the final-linear quarters
            ocb = pers.tile([128, 512], BF16, tag="ocb")
            nc.scalar.copy(ocb[0:64, :], acc0[0:64, :])
            nc.vector.tensor_copy(ocb[64:128, :], acc1[0:64, :])
            dnx = pers.tile([1, 1024], F32, tag="dnx")
            nc.scalar.copy(dnx[0:1, 0:512], acc0[64:65, :])
            nc.vector.tensor_copy(dnx[0:1, 512:1024], acc1[64:65, :])
            nc.sync.dma_start(DN.ap(), dnx[:])
            for b in range(8):
                pb = scp.tile([128, 1024], F32, tag="sc")
                nc.tensor.matmul(pb[:, 0:512], U_sb[0:64, ts(b, 128)], ocb[0:64, :])
                nc.tensor.matmul(
                    pb[:, 512:1024], U_sb[64:128, ts(b, 128)], ocb[64:128, :]
                )
                st = stg.tile([128, 1024], BF16, tag="stb")
                if b % 2 == 0:
                    nc.scalar.copy(st[:, 0:512], pb[:, 0:512])
                    nc.vector.tensor_copy(st[:, 512:1024], pb[:, 512:1024])
                else:
                    nc.vector.tensor_copy(st[:, 0:512], pb[:, 0:512])
                    nc.scalar.copy(st[:, 512:1024], pb[:, 512:1024])
                nc.sync.dma_start(PTB.ap()[ts(b, 128), :], st[:])

    nc.compile()
    return nc


def _get_nc():
    if "nc" not in _CACHE:
        _CACHE["nc"] = _build_nc()
    return _CACHE["nc"]


def make_in_maps(Q, K, V, Wq, bq, Wk, bk, Wv, bv, Wo, bo):
    bf = ml_dtypes.bfloat16
    e4 = ml_dtypes.float8_e4m3
    in_maps = []
    for i in range(NCORES):
        c0 = i * CW
        h0, h1 = 2 * i, 2 * i + 1

        G_pack = np.zeros((CW, 64), np.float32)
        gq = np.zeros((CW, 1), np.float32)
        U_pack = np.zeros((CW, DK), np.float32)
        for hh, h in enumerate((h0, h1)):
            G_pack[hh * 64 : (hh + 1) * 64, :] = (Wq[h].T @ Wk[h]) / 8.0
            gq[hh * 64 : (hh + 1) * 64, 0] = (Wk[h].T @ bq[h]) / 8.0
            # U_pack[h*64+e, :] = (Wo_slice @ Wv[h])[:, e]
            U_pack[hh * 64 : (hh + 1) * 64, :] = (
                Wo[:, h * DH : (h + 1) * DH] @ Wv[h]
            ).T

        # VP layout per chunk-pair p: 2 k-tiles x 160 cols, each k-tile =
        # [h0 V 64 | ones | pad 15 | h1 V 64 | ones | pad 15]
        VP = np.zeros((CW, (NCH // 2) * VPW), np.float32)
        for p in range(NCH // 2):
            for i_t in range(2):
                j = 2 * p + i_t
                rows = slice(j * 128, (j + 1) * 128)
                for hh in range(2):
                    cbase = p * VPW + i_t * 160 + hh * 80
                    VP[:, cbase : cbase + 64] = V[rows, c0 + hh * 64 : c0 + (hh + 1) * 64]
                    VP[:, cbase + 64] = 1.0

        in_maps.append(
            {
                "QTd": np.ascontiguousarray(Q[:, c0 : c0 + CW].T).astype(bf),
                "KTd": np.ascontiguousarray(K[:, c0 : c0 + CW].T).astype(bf),
                "VPd": VP.astype(e4),
                "Gd": G_pack.astype(bf),
                "Ud": U_pack.astype(bf),
                "Gq": gq.astype(np.float32),
            }
        )
    return in_maps


def kernel(Q, K, V, Wq, bq, Wk, bk, Wv, bv, Wo, bo, _spmd_kwargs=None):
    Q, K, V = (np.asarray(x, np.float32) for x in (Q, K, V))
    Wq, bq, Wk, bk, Wv, bv = (
        np.asarray(x, np.float32) for x in (Wq, bq, Wk, bk, Wv, bv)
    )
    Wo, bo = np.asarray(Wo, np.float32), np.asarray(bo, np.float32)

    nc = _get_nc()
    in_maps = make_in_maps(Q, K, V, Wq, bq, Wk, bk, Wv, bv, Wo, bo)
    res = run_bass_kernel_spmd(
        nc, in_maps, core_ids=list(range(NCORES)), **(_spmd_kwargs or {})
    )

    acc = np.zeros((DK, S), np.float64)
    for i in range(NCORES):
        r = res.results[i]
        # PT cols 1536:2048 are never written on-device (PTB replaces them)
        acc[:, 0:1536] += r["PT"].astype(np.float64)[:, 0:1536]
        dn = r["DN"].astype(np.float64)[0]
        ptb = r["PTB"].astype(np.float64)
        acc[:, 1536:2048] += ptb[:, 0:512] / dn[0:512][None, :]
        acc[:, 1536:2048] += ptb[:, 512:1024] / dn[512:1024][None, :]
    bo_eff = bo + Wo @ bv.reshape(-1)
    out = (acc.T + bo_eff).astype(np.float32)
    if _spmd_kwargs:
        return out, res
    return out


# revision 19
# speedup vs baseline: 1.2632x; 1.2632x over previous
"""Multi-head attention (16 heads, d_model=1024, S=2048) on 8 Trainium2 cores.

Tensor-parallel over heads: each core owns 2 heads (its 128-feature slice).
The host sums the 8 row-parallel partials and adds the folded bias.

v2 design (vs the v1 baseline's 126us):
  - Algebraic folds kill both projections:
      scores = (Q Wq.T + bq)(K Wk.T).T  ==  (Q G + g) K.T
        with G = Wq.T Wk / 8,  g = Wk.T bq / 8  (host-precomputed, bf16)
      o @ Wo.T  ==  (A V_raw) @ U.T  with U = Wo_slice Wv (host, bf16)
    so the kernel runs ONE projection (qm = G.T @ QT + g) and uses RAW
    K chunks as the scores stationaries and RAW V chunks (host-quantized
    e4m3, with ones columns for the softmax denominators) as the attn@V
    stationaries.  bv folds into bo on the host; bk drops (softmax shift
    invariance).
  - mm2 (scores): per-head K=64 row-tiled matmul pairs run CONCURRENTLY
    in disjoint PE row-groups (measured 119ns per N=512 MM vs 220 solo).
  - mm3 (attn@V): fp8e4 DoubleRow packs chunk-pairs (2x contraction per
    instruction, measured 218ns per N=512 MM = 2 chunks of work).
  - exp: e4m3 softmax weights; ACT does true Exp for 9/16 chunks per
    segment, DVE does a one-op Schraudolph (e4m3 bits = s*A + B as uint8,
    HW rounds) for 7/16.  Only ACT/DVE can read PSUM (gpsimd cannot),
    which makes PSUM evacuation the wall this design is balanced around.
  - Softmax normalize: denominators ride the ones columns into row 64 of
    the fp32 accumulators; reciprocal via the DMA spread/broadcast trick;
    the oT multiplies land on gpsimd/DVE.
  - Final linear: U-stationary, 8 N=512 quarters per 512-col segment,
    woven one segment behind the attention loop so output DMA streams.
"""

import sys

for _p in ("/opt/trn_rl_repo",):
    if _p not in sys.path:
        sys.path.insert(0, _p)

from contextlib import ExitStack

import ml_dtypes
import numpy as np

import concourse.bass as bass
import concourse.tile as tile
from concourse import bacc, mybir
from concourse.bass import ts
from concourse.bass_utils import run_bass_kernel_spmd

S = 2048          # sequence length
DK = 1024         # d_model
H = 16            # heads
DH = 64           # head dim
NCORES = 8
CW = 128          # per-core feature slice (2 heads x 64)
NCH = 16          # 128-row key chunks
NSEG = 4          # 512-col query segments
VPW = 320         # bytes per chunk-pair in VP: 2 k-tiles x (64 v + 1 + 15 pad)

F32 = mybir.dt.float32
BF16 = mybir.dt.bfloat16
E4 = mybir.dt.float8e4
U8 = mybir.dt.uint8
EXP = mybir.ActivationFunctionType.Exp
DR = mybir.MatmulPerfMode.DoubleRow

# e4m3-bits Schraudolph: bits8(exp(s)) ~= round(s * A + B) (HW rounds on
# the fp32->u8 convert; B tuned for min max-rel-err over the normal range)
SCH_A = float(np.float32(8.0 * np.log2(np.e)))
SCH_B = 55.62
# chunks per segment whose exp runs on ACT (rest on DVE Schraudolph)
ACT_SET = frozenset((0, 2, 4, 6, 8, 9, 11, 13, 15))

N_WARMUP_MM = 5

_CACHE = {}


def _build_nc():
    nc = bacc.Bacc(
        "TRN2", target_bir_lowering=False, debug=False, enable_asserts=False
    )

    QTd = nc.dram_tensor("QTd", [CW, S], BF16, kind="ExternalInput")
    KTd = nc.dram_tensor("KTd", [CW, S], BF16, kind="ExternalInput")
    # raw V chunk-pair stationaries (e4m3) with ones cols at m=64 per head
    VPd = nc.dram_tensor("VPd", [CW, (NCH // 2) * VPW], E4, kind="ExternalInput")
    Gd = nc.dram_tensor("Gd", [CW, 64], BF16, kind="ExternalInput")
    Ud = nc.dram_tensor("Ud", [CW, DK], BF16, kind="ExternalInput")
    Gq = nc.dram_tensor("Gq", [CW, 1], F32, kind="ExternalInput")
    PT = nc.dram_tensor("PT", [DK, S], BF16, kind="ExternalOutput")
    # last segment exported unnormalized, per head; host divides by DN
    PTB = nc.dram_tensor("PTB", [DK, 1024], BF16, kind="ExternalOutput")
    DN = nc.dram_tensor("DN", [1, 1024], F32, kind="ExternalOutput")

    with tile.TileContext(nc) as tc:
        with ExitStack() as ctx:
            pers = ctx.enter_context(tc.tile_pool(name="pers", bufs=1))
            expool = ctx.enter_context(tc.tile_pool(name="expool", bufs=3))
            stg = ctx.enter_context(tc.tile_pool(name="stg", bufs=4))
            nrm = ctx.enter_context(tc.tile_pool(name="nrm", bufs=2))
            scp = ctx.enter_context(tc.tile_pool(name="scp", bufs=3, space="PSUM"))
            accp = ctx.enter_context(tc.tile_pool(name="accp", bufs=1, space="PSUM"))
            dscr = ctx.enter_context(tc.tile_pool(name="dscr", bufs=2, space="DRAM"))

            # ---- t=0: ACT exp table preload + PE warmup fodder ----
            warm = pers.tile([128, 512], BF16, tag="warm")
            nc.vector.memset(warm[:], 0.0)
            wex = pers.tile([128, 1], F32, tag="wex")
            nc.scalar.activation(wex[:], warm[:, 0:1], EXP)

            # ---- input loads on the two HW-DGE queues ----
            KT = pers.tile([128, S], BF16, tag="KT")
            QT = pers.tile([128, S], BF16, tag="QT")
            VP = pers.tile([128, (NCH // 2) * VPW], E4, tag="VP")
            G_sb = pers.tile([128, 64], BF16, tag="G")
            U_sb = pers.tile([128, DK], BF16, tag="U")
            gq_sb = pers.tile([128, 1], F32, tag="gq")

            nc.sync.dma_start(G_sb[:], Gd.ap())
            nc.sync.dma_start(gq_sb[:], Gq.ap())
            nc.scalar.dma_start(QT[:, ts(0, 1024)], QTd.ap()[:, ts(0, 1024)])
            nc.sync.dma_start(KT[:, ts(0, 1024)], KTd.ap()[:, ts(0, 1024)])
            nc.sync.dma_start(VP[:], VPd.ap())
            nc.sync.dma_start(KT[:, ts(1, 1024)], KTd.ap()[:, ts(1, 1024)])
            nc.scalar.dma_start(QT[:, ts(1, 1024)], QTd.ap()[:, ts(1, 1024)])
            nc.scalar.dma_start(U_sb[:], Ud.ap())

            # ---- PE warmup stream (HAM ramp; no readers) ----
            for _ in range(N_WARMUP_MM):
                pw = scp.tile([128, 1024], F32, tag="sc")
                nc.tensor.matmul(pw[:, 0:512], warm[:, 0:128], warm[:])

            # ---- qm projection: qmT = G.T @ QT (+ g), row-tiled pairs;
            # only slice 0 gates segment 0 -- slices 1-3 weave into it ----
            qm = pers.tile([128, S], BF16, tag="qm")

            def qm_slice(sl):
                def emit():
                    pq = scp.tile([128, 1024], F32, tag="sc")
                    nc.tensor.matmul(
                        pq[0:64, 0:512], G_sb[0:64, :], QT[0:64, ts(sl, 512)]
                    )
                    nc.tensor.matmul(
                        pq[64:128, 0:512], G_sb[64:128, :], QT[64:128, ts(sl, 512)]
                    )
                    nc.vector.tensor_scalar_add(
                        qm[:, ts(sl, 512)], pq[:, 0:512], gq_sb[:]
                    )

                return emit

            qm_slice(0)()

            # ---- attention + woven final linear ----
            oT = pers.tile([128, S], BF16, tag="oT")

            def vp_view(p, h):
                return VP[:, p * VPW : (p + 1) * VPW].rearrange(
                    "q (i c) -> q i c", i=2
                )[:, :, h * 80 : h * 80 + 65]

            def fl_unit(s, u):
                """Final-linear super-unit: row-blocks 2u, 2u+1 -> one
                [128,1024] psum tile -> one staged copy -> one DMA."""

                def emit():
                    p = scp.tile([128, 1024], F32, tag="sc")
                    nc.tensor.matmul(
                        p[:, 0:512], U_sb[:, ts(2 * u, 128)], oT[:, ts(s, 512)]
                    )
                    nc.tensor.matmul(
                        p[:, 512:1024], U_sb[:, ts(2 * u + 1, 128)], oT[:, ts(s, 512)]
                    )
                    st = stg.tile([128, 1024], BF16, tag="st")
                    if u % 2 == 0:
                        nc.scalar.copy(st[:], p[:])
                    else:
                        nc.vector.tensor_copy(st[:], p[:])
                    nc.sync.dma_start(
                        PT.ap()[
                            2 * u * 128 : (2 * u + 2) * 128,
                            s * 512 : (s + 1) * 512,
                        ].rearrange("(i p) f -> p i f", i=2),
                        st[:].rearrange("p (i f) -> p i f", i=2),
                    )

                return emit

            # per-seg normalize state handed to the next segment
            ACT_EXP = frozenset((0, 2, 4, 6, 8, 10, 12, 14, 15))

            def segment(s, extras=(), pre=(), normalize=True):
                """pre: closures popped early (seg0's deferred qm slices);
                extras: the previous segment's final-linear units, emitted
                at the boundary after the mm3 flush."""
                extras = list(extras)
                pre = list(pre)
                acc0 = accp.tile([65, 512], F32, tag="acc0")
                acc1 = accp.tile([65, 512], F32, tag="acc1")
                acc = [acc0, acc1]
                ep = None
                mm3q = []

                def mm3_maker(p, ep_):
                    def emit():
                        for h in (0, 1):
                            nc.tensor.matmul(
                                acc[h][:],
                                vp_view(p, h),
                                ep_[:, :, h * 512 : (h + 1) * 512],
                                start=(p == 0),
                                stop=(p == 7),
                                perf_mode=DR,
                            )

                    return emit

                for j in range(NCH):
                    sc = scp.tile([128, 1024], F32, tag="sc")
                    nc.tensor.matmul(
                        sc[:, 0:512], KT[0:64, ts(j, 128)], qm[0:64, ts(s, 512)]
                    )
                    nc.tensor.matmul(
                        sc[:, 512:1024], KT[64:128, ts(j, 128)], qm[64:128, ts(s, 512)]
                    )
                    if j % 2 == 1 and len(mm3q) == 2:
                        mm3q.pop(0)()
                    if j % 2 == 0:
                        ep = expool.tile([128, 2, 1024], E4, tag="ep")
                    esl = ep[:, j % 2, :]
                    if j in ACT_EXP:
                        nc.scalar.activation(esl, sc[:], EXP)
                    else:
                        nc.vector.tensor_scalar(
                            esl.bitcast(U8), sc[:], SCH_A, SCH_B,
                            op0=mybir.AluOpType.mult, op1=mybir.AluOpType.add,
                        )
                    if j % 2 == 1:
                        mm3q.append(mm3_maker(j // 2, ep))
                    if j in (2, 4, 6) and pre:
                        pre.pop(0)()

                # boundary: flush lagged mm3 pairs, then the previous
                # segment's final-linear units (their oT has been ready for
                # a whole segment -- the PE never waits here)
                mm3q.pop(0)()
                mm3q.pop(0)()
                while extras:
                    extras.pop(0)()
                if not normalize:
                    return acc0, acc1

                # normalize (engine-cheap spread form; the consumers of oT
                # run a full segment later, so chain latency is free):
                # oc copies -> den spread via DRAM -> [64,16] reciprocal ->
                # broadcast back -> gpsimd multiplies
                oc0 = nrm.tile([65, 512], F32, tag="oc0")
                nc.scalar.copy(oc0[:], acc0[:])
                oc1 = nrm.tile([65, 512], F32, tag="oc1")
                nc.vector.tensor_copy(oc1[:], acc1[:])
                dn = dscr.tile([1, 1024], F32, tag="dn")
                nc.sync.dma_start(dn[0:1, 0:512], oc0[64:65, :])
                nc.sync.dma_start(dn[0:1, 512:1024], oc1[64:65, :])
                d8 = nrm.tile([64, 16], F32, tag="d8")
                nc.sync.dma_start(
                    d8[:, 0:8], dn[0:1, 0:512].rearrange("a (p f) -> (a p) f", p=64)
                )
                nc.sync.dma_start(
                    d8[:, 8:16],
                    dn[0:1, 512:1024].rearrange("a (p f) -> (a p) f", p=64),
                )
                r8 = nrm.tile([64, 16], F32, tag="r8")
                nc.vector.reciprocal(r8[:], d8[:])
                rn = dscr.tile([1, 1024], F32, tag="rn")
                nc.sync.dma_start(
                    rn[0:1, 0:512].rearrange("a (p f) -> (a p) f", p=64), r8[:, 0:8]
                )
                nc.sync.dma_start(
                    rn[0:1, 512:1024].rearrange("a (p f) -> (a p) f", p=64),
                    r8[:, 8:16],
                )
                rb0 = nrm.tile([64, 512], F32, tag="rb0")
                nc.sync.dma_start(rb0[:], rn[0:1, 0:512].to_broadcast((64, 512)))
                rb1 = nrm.tile([64, 512], F32, tag="rb1")
                nc.sync.dma_start(rb1[:], rn[0:1, 512:1024].to_broadcast((64, 512)))
                nc.gpsimd.tensor_mul(oT[0:64, ts(s, 512)], oc0[0:64, :], rb0[:])
                nc.gpsimd.tensor_mul(oT[64:128, ts(s, 512)], oc1[0:64, :], rb1[:])

                return None

            segment(0, pre=[qm_slice(1), qm_slice(2), qm_slice(3)])
            segment(1, extras=[fl_unit(0, u) for u in range(4)])
            segment(2, extras=[fl_unit(1, u) for u in range(4)])
            acc0, acc1 = segment(
                3, extras=[fl_unit(2, u) for u in range(4)], normalize=False
            )
            # tail: export seg 3 unnormalized per head (host divides by DN) --
            # no normalize chain gates the final-linear quarters
            ocb = pers.tile([128, 512], BF16, tag="ocb")
            nc.scalar.copy(ocb[0:64, :], acc0[0:64, :])
            nc.vector.tensor_copy(ocb[64:128, :], acc1[0:64, :])
            dnx = pers.tile([1, 1024], F32, tag="dnx")
            nc.scalar.copy(dnx[0:1, 0:512], acc0[64:65, :])
            nc.vector.tensor_copy(dnx[0:1, 512:1024], acc1[64:65, :])
            nc.sync.dma_start(DN.ap(), dnx[:])
            for b in range(8):
                pb = scp.tile([128, 1024], F32, tag="sc")
                nc.tensor.matmul(pb[:, 0:512], U_sb[0:64, ts(b, 128)], ocb[0:64, :])
                nc.tensor.matmul(
                    pb[:, 512:1024], U_sb[64:128, ts(b, 128)], ocb[64:128, :]
                )
                st = stg.tile([128, 1024], BF16, tag="stb")
                if b % 2 == 0:
                    nc.scalar.copy(st[:, 0:512], pb[:, 0:512])
                    nc.vector.tensor_copy(st[:, 512:1024], pb[:, 512:1024])
                else:
                    nc.vector.tensor_copy(st[:, 0:512], pb[:, 0:512])
                    nc.scalar.copy(st[:, 512:1024], pb[:, 512:1024])
                nc.sync.dma_start(PTB.ap()[ts(b, 128), :], st[:])

    nc.compile()
    return nc


def _get_nc():
    if "nc" not in _CACHE:
        _CACHE["nc"] = _build_nc()
    return _CACHE["nc"]


def make_in_maps(Q, K, V, Wq, bq, Wk, bk, Wv, bv, Wo, bo):
    bf = ml_dtypes.bfloat16
    e4 = ml_dtypes.float8_e4m3
    in_maps = []
    for i in range(NCORES):
        c0 = i * CW
        h0, h1 = 2 * i, 2 * i + 1

        G_pack = np.zeros((CW, 64), np.float32)
        gq = np.zeros((CW, 1), np.float32)
        U_pack = np.zeros((CW, DK), np.float32)
        for hh, h in enumerate((h0, h1)):
            G_pack[hh * 64 : (hh + 1) * 64, :] = (Wq[h].T @ Wk[h]) / 8.0
            gq[hh * 64 : (hh + 1) * 64, 0] = (Wk[h].T @ bq[h]) / 8.0
            # U_pack[h*64+e, :] = (Wo_slice @ Wv[h])[:, e]
            U_pack[hh * 64 : (hh + 1) * 64, :] = (
                Wo[:, h * DH : (h + 1) * DH] @ Wv[h]
            ).T

        # VP layout per chunk-pair p: 2 k-tiles x 160 cols, each k-tile =
        # [h0 V 64 | ones | pad 15 | h1 V 64 | ones | pad 15]
        VP = np.zeros((CW, (NCH // 2) * VPW), np.float32)
        for p in range(NCH // 2):
            for i_t in range(2):
                j = 2 * p + i_t
                rows = slice(j * 128, (j + 1) * 128)
                for hh in range(2):
                    cbase = p * VPW + i_t * 160 + hh * 80
                    VP[:, cbase : cbase + 64] = V[rows, c0 + hh * 64 : c0 + (hh + 1) * 64]
                    VP[:, cbase + 64] = 1.0

        in_maps.append(
            {
                "QTd": np.ascontiguousarray(Q[:, c0 : c0 + CW].T).astype(bf),
                "KTd": np.ascontiguousarray(K[:, c0 : c0 + CW].T).astype(bf),
                "VPd": VP.astype(e4),
                "Gd": G_pack.astype(bf),
                "Ud": U_pack.astype(bf),
                "Gq": gq.astype(np.float32),
            }
        )
    return in_maps


def kernel(Q, K, V, Wq, bq, Wk, bk, Wv, bv, Wo, bo, _spmd_kwargs=None):
    Q, K, V = (np.asarray(x, np.float32) for x in (Q, K, V))
    Wq, bq, Wk, bk, Wv, bv = (
        np.asarray(x, np.float32) for x in (Wq, bq, Wk, bk, Wv, bv)
    )
    Wo, bo = np.asarray(Wo, np.float32), np.asarray(bo, np.float32)

    nc = _get_nc()
    in_maps = make_in_maps(Q, K, V, Wq, bq, Wk, bk, Wv, bv, Wo, bo)
    res = run_bass_kernel_spmd(
        nc, in_maps, core_ids=list(range(NCORES)), **(_spmd_kwargs or {})
    )

    acc = np.zeros((DK, S), np.float64)
    for i in range(NCORES):
        r = res.results[i]
        # PT cols 1536:2048 are never written on-device (PTB replaces them)
        acc[:, 0:1536] += r["PT"].astype(np.float64)[:, 0:1536]
        dn = r["DN"].astype(np.float64)[0]
        ptb = r["PTB"].astype(np.float64)
        acc[:, 1536:2048] += ptb[:, 0:512] / dn[0:512][None, :]
        acc[:, 1536:2048] += ptb[:, 512:1024] / dn[512:1024][None, :]
    bo_eff = bo + Wo @ bv.reshape(-1)
    out = (acc.T + bo_eff).astype(np.float32)
    if _spmd_kwargs:
        return out, res
    return out


# revision 20
# speedup vs baseline: 1.4362x; 1.1369x over previous
"""Multi-head attention (16 heads, d_model=1024, S=2048) on 8 Trainium2 cores.

Tensor-parallel over heads: each core owns 2 heads (its 128-feature slice).
The host sums the 8 row-parallel partials and adds the folded bias.

v2 design (vs the v1 baseline's 126us):
  - Algebraic folds kill both projections:
      scores = (Q Wq.T + bq)(K Wk.T).T  ==  (Q G + g) K.T
        with G = Wq.T Wk / 8,  g = Wk.T bq / 8  (host-precomputed, bf16)
      o @ Wo.T  ==  (A V_raw) @ U.T  with U = Wo_slice Wv (host, bf16)
    so the kernel runs ONE projection (qm = G.T @ QT + g) and uses RAW
    K chunks as the scores stationaries and RAW V chunks (host-quantized
    e4m3, with ones columns for the softmax denominators) as the attn@V
    stationaries.  bv folds into bo on the host; bk drops (softmax shift
    invariance).
  - mm2 (scores): per-head K=64 row-tiled matmul pairs run CONCURRENTLY
    in disjoint PE row-groups (measured 119ns per N=512 MM vs 220 solo).
  - mm3 (attn@V): fp8e4 DoubleRow packs chunk-pairs (2x contraction per
    instruction, measured 218ns per N=512 MM = 2 chunks of work).
  - exp: e4m3 softmax weights; ACT does true Exp for 9/16 chunks per
    segment, DVE does a one-op Schraudolph (e4m3 bits = s*A + B as uint8,
    HW rounds) for 7/16.  Only ACT/DVE can read PSUM (gpsimd cannot),
    which makes PSUM evacuation the wall this design is balanced around.
  - Softmax normalize: denominators ride the ones columns into row 64 of
    the fp32 accumulators; reciprocal via the DMA spread/broadcast trick;
    the oT multiplies land on gpsimd/DVE.
  - Final linear: U-stationary, 8 N=512 quarters per 512-col segment,
    woven one segment behind the attention loop so output DMA streams.
"""

import sys

for _p in ("/opt/trn_rl_repo",):
    if _p not in sys.path:
        sys.path.insert(0, _p)

from contextlib import ExitStack

import ml_dtypes
import numpy as np

import concourse.bass as bass
import concourse.tile as tile
from concourse import bacc, mybir
from concourse.bass import ts
from concourse.bass_utils import run_bass_kernel_spmd

S = 2048          # sequence length
DK = 1024         # d_model
H = 16            # heads
DH = 64           # head dim
NCORES = 8
CW = 128          # per-core feature slice (2 heads x 64)
NCH = 16          # 128-row key chunks
NSEG = 4          # 512-col query segments
VPW = 320         # bytes per chunk-pair in VP: 2 k-tiles x (64 v + 1 + 15 pad)

F32 = mybir.dt.float32
BF16 = mybir.dt.bfloat16
E4 = mybir.dt.float8e4
U8 = mybir.dt.uint8
EXP = mybir.ActivationFunctionType.Exp
DR = mybir.MatmulPerfMode.DoubleRow

# e4m3-bits Schraudolph: bits8(exp(s)) ~= round(s * A + B) (HW rounds on
# the fp32->u8 convert; B tuned for min max-rel-err over the normal range)
SCH_A = float(np.float32(8.0 * np.log2(np.e)))
SCH_B = 55.62
# chunks per segment whose exp runs on ACT (rest on DVE Schraudolph)
ACT_SET = frozenset((0, 2, 4, 6, 8, 9, 11, 13, 15))

N_WARMUP_MM = 5

_CACHE = {}


def _build_nc():
    nc = bacc.Bacc(
        "TRN2", target_bir_lowering=False, debug=False, enable_asserts=False
    )

    QTd = nc.dram_tensor("QTd", [CW, S], BF16, kind="ExternalInput")
    KTd = nc.dram_tensor("KTd", [CW, S], BF16, kind="ExternalInput")
    # raw V chunk-pair stationaries (e4m3) with ones cols at m=64 per head
    VPd = nc.dram_tensor("VPd", [CW, (NCH // 2) * VPW], E4, kind="ExternalInput")
    Gd = nc.dram_tensor("Gd", [CW, 64], BF16, kind="ExternalInput")
    Ud = nc.dram_tensor("Ud", [CW, DK], BF16, kind="ExternalInput")
    Gq = nc.dram_tensor("Gq", [CW, 1], F32, kind="ExternalInput")
    PT = nc.dram_tensor("PT", [DK, S], BF16, kind="ExternalOutput")
    # last segment exported unnormalized, per head; host divides by DN
    PTB = nc.dram_tensor("PTB", [DK, 1024], BF16, kind="ExternalOutput")
    DN = nc.dram_tensor("DN", [1, 1024], F32, kind="ExternalOutput")

    with tile.TileContext(nc) as tc:
        with ExitStack() as ctx:
            pers = ctx.enter_context(tc.tile_pool(name="pers", bufs=1))
            expool = ctx.enter_context(tc.tile_pool(name="expool", bufs=3))
            stg = ctx.enter_context(tc.tile_pool(name="stg", bufs=4))
            nrm = ctx.enter_context(tc.tile_pool(name="nrm", bufs=2))
            scp = ctx.enter_context(tc.tile_pool(name="scp", bufs=3, space="PSUM"))
            accp = ctx.enter_context(tc.tile_pool(name="accp", bufs=1, space="PSUM"))
            dscr = ctx.enter_context(tc.tile_pool(name="dscr", bufs=2, space="DRAM"))

            # ---- t=0: ACT exp table preload + PE warmup fodder ----
            warm = pers.tile([128, 512], BF16, tag="warm")
            nc.vector.memset(warm[:], 0.0)
            wex = pers.tile([128, 1], F32, tag="wex")
            nc.scalar.activation(wex[:], warm[:, 0:1], EXP)

            # ---- input loads on the two HW-DGE queues ----
            KT = pers.tile([128, S], BF16, tag="KT")
            QT = pers.tile([128, S], BF16, tag="QT")
            VP = pers.tile([128, (NCH // 2) * VPW], E4, tag="VP")
            G_sb = pers.tile([128, 64], BF16, tag="G")
            U_sb = pers.tile([128, DK], BF16, tag="U")
            gq_sb = pers.tile([128, 1], F32, tag="gq")

            nc.sync.dma_start(G_sb[:], Gd.ap())
            nc.sync.dma_start(gq_sb[:], Gq.ap())
            nc.scalar.dma_start(QT[:, ts(0, 1024)], QTd.ap()[:, ts(0, 1024)])
            nc.sync.dma_start(KT[:, ts(0, 1024)], KTd.ap()[:, ts(0, 1024)])
            nc.sync.dma_start(VP[:], VPd.ap())
            nc.sync.dma_start(KT[:, ts(1, 1024)], KTd.ap()[:, ts(1, 1024)])
            nc.scalar.dma_start(QT[:, ts(1, 1024)], QTd.ap()[:, ts(1, 1024)])
            nc.scalar.dma_start(U_sb[:], Ud.ap())

            # ---- PE warmup stream (HAM ramp; no readers) ----
            for _ in range(N_WARMUP_MM):
                pw = scp.tile([128, 1024], F32, tag="sc")
                nc.tensor.matmul(pw[:, 0:512], warm[:, 0:128], warm[:])

            # ---- qm projection: qmT = G.T @ QT (+ g), row-tiled pairs;
            # only slice 0 gates segment 0 -- slices 1-3 weave into it ----
            qm = pers.tile([128, S], BF16, tag="qm")

            def qm_slice(sl):
                def emit():
                    pq = scp.tile([128, 1024], F32, tag="sc")
                    nc.tensor.matmul(
                        pq[0:64, 0:512], G_sb[0:64, :], QT[0:64, ts(sl, 512)]
                    )
                    nc.tensor.matmul(
                        pq[64:128, 0:512], G_sb[64:128, :], QT[64:128, ts(sl, 512)]
                    )
                    nc.vector.tensor_scalar_add(
                        qm[:, ts(sl, 512)], pq[:, 0:512], gq_sb[:]
                    )

                return emit

            qm_slice(0)()

            # ---- attention + woven final linear ----
            oT = pers.tile([128, S], BF16, tag="oT")

            def vp_view(p, h):
                return VP[:, p * VPW : (p + 1) * VPW].rearrange(
                    "q (i c) -> q i c", i=2
                )[:, :, h * 80 : h * 80 + 65]

            def fl_unit(s, u):
                """Final-linear super-unit: row-blocks 2u, 2u+1 -> one
                [128,1024] psum tile -> one staged copy -> one DMA."""

                def emit():
                    p = scp.tile([128, 1024], F32, tag="sc")
                    nc.tensor.matmul(
                        p[:, 0:512], U_sb[:, ts(2 * u, 128)], oT[:, ts(s, 512)]
                    )
                    nc.tensor.matmul(
                        p[:, 512:1024], U_sb[:, ts(2 * u + 1, 128)], oT[:, ts(s, 512)]
                    )
                    st = stg.tile([128, 1024], BF16, tag="st")
                    if u % 2 == 0:
                        nc.scalar.copy(st[:], p[:])
                    else:
                        nc.vector.tensor_copy(st[:], p[:])
                    nc.sync.dma_start(
                        PT.ap()[
                            2 * u * 128 : (2 * u + 2) * 128,
                            s * 512 : (s + 1) * 512,
                        ].rearrange("(i p) f -> p i f", i=2),
                        st[:].rearrange("p (i f) -> p i f", i=2),
                    )

                return emit

            # per-seg normalize state handed to the next segment
            ACT_EXP = frozenset((0, 2, 4, 6, 8, 10, 12, 14, 15))

            def segment(s, extras=(), pre=(), normalize=True):
                """pre: closures popped early (seg0's deferred qm slices);
                extras: the previous segment's final-linear units, emitted
                at the boundary after the mm3 flush."""
                extras = list(extras)
                pre = list(pre)
                acc0 = accp.tile([65, 512], F32, tag="acc0")
                acc1 = accp.tile([65, 512], F32, tag="acc1")
                acc = [acc0, acc1]
                ep = None
                mm3q = []

                def mm3_maker(p, ep_):
                    def emit():
                        for h in (0, 1):
                            nc.tensor.matmul(
                                acc[h][:],
                                vp_view(p, h),
                                ep_[:, :, h * 512 : (h + 1) * 512],
                                start=(p == 0),
                                stop=(p == 7),
                                perf_mode=DR,
                            )

                    return emit

                for j in range(NCH):
                    sc = scp.tile([128, 1024], F32, tag="sc")
                    nc.tensor.matmul(
                        sc[:, 0:512], KT[0:64, ts(j, 128)], qm[0:64, ts(s, 512)]
                    )
                    nc.tensor.matmul(
                        sc[:, 512:1024], KT[64:128, ts(j, 128)], qm[64:128, ts(s, 512)]
                    )
                    if j % 2 == 1 and len(mm3q) == 2:
                        mm3q.pop(0)()
                    if j % 2 == 0:
                        ep = expool.tile([128, 2, 1024], E4, tag="ep")
                    esl = ep[:, j % 2, :]
                    if j in ACT_EXP:
                        nc.scalar.activation(esl, sc[:], EXP)
                    else:
                        nc.vector.tensor_scalar(
                            esl.bitcast(U8), sc[:], SCH_A, SCH_B,
                            op0=mybir.AluOpType.mult, op1=mybir.AluOpType.add,
                        )
                    if j % 2 == 1:
                        mm3q.append(mm3_maker(j // 2, ep))
                    if j in (2, 4, 6) and pre:
                        pre.pop(0)()

                # boundary: flush lagged mm3 pairs, then the previous
                # segment's final-linear units (their oT has been ready for
                # a whole segment -- the PE never waits here)
                mm3q.pop(0)()
                mm3q.pop(0)()
                while extras:
                    extras.pop(0)()
                if not normalize:
                    return acc0, acc1

                # normalize (engine-cheap spread form; the consumers of oT
                # run a full segment later, so chain latency is free):
                # oc copies -> den spread via DRAM -> [64,16] reciprocal ->
                # broadcast back -> gpsimd multiplies
                oc0 = nrm.tile([65, 512], F32, tag="oc0")
                nc.scalar.copy(oc0[:], acc0[:])
                oc1 = nrm.tile([65, 512], F32, tag="oc1")
                nc.vector.tensor_copy(oc1[:], acc1[:])
                dn = dscr.tile([1, 1024], F32, tag="dn")
                nc.sync.dma_start(dn[0:1, 0:512], oc0[64:65, :])
                nc.sync.dma_start(dn[0:1, 512:1024], oc1[64:65, :])
                d8 = nrm.tile([64, 16], F32, tag="d8")
                nc.sync.dma_start(
                    d8[:, 0:8], dn[0:1, 0:512].rearrange("a (p f) -> (a p) f", p=64)
                )
                nc.sync.dma_start(
                    d8[:, 8:16],
                    dn[0:1, 512:1024].rearrange("a (p f) -> (a p) f", p=64),
                )
                r8 = nrm.tile([64, 16], F32, tag="r8")
                nc.vector.reciprocal(r8[:], d8[:])
                rn = dscr.tile([1, 1024], F32, tag="rn")
                nc.sync.dma_start(
                    rn[0:1, 0:512].rearrange("a (p f) -> (a p) f", p=64), r8[:, 0:8]
                )
                nc.sync.dma_start(
                    rn[0:1, 512:1024].rearrange("a (p f) -> (a p) f", p=64),
                    r8[:, 8:16],
                )
                rb0 = nrm.tile([64, 512], F32, tag="rb0")
                nc.sync.dma_start(rb0[:], rn[0:1, 0:512].to_broadcast((64, 512)))
                rb1 = nrm.tile([64, 512], F32, tag="rb1")
                nc.sync.dma_start(rb1[:], rn[0:1, 512:1024].to_broadcast((64, 512)))
                nc.gpsimd.tensor_mul(oT[0:64, ts(s, 512)], oc0[0:64, :], rb0[:])
                nc.gpsimd.tensor_mul(oT[64:128, ts(s, 512)], oc1[0:64, :], rb1[:])

                return None

            segment(0, pre=[qm_slice(1), qm_slice(2), qm_slice(3)])
            segment(1)
            segment(2, extras=[fl_unit(0, u) for u in range(4)])
            acc0, acc1 = segment(
                3,
                extras=[fl_unit(1, u) for u in range(4)]
                + [fl_unit(2, u) for u in range(4)],
                normalize=False,
            )
            # tail: export seg 3 unnormalized per head (host divides by DN) --
            # no normalize chain gates the final-linear quarters
            ocb = pers.tile([128, 512], BF16, tag="ocb")
            nc.scalar.copy(ocb[0:64, :], acc0[0:64, :])
            nc.vector.tensor_copy(ocb[64:128, :], acc1[0:64, :])
            dnx = pers.tile([1, 1024], F32, tag="dnx")
            nc.scalar.copy(dnx[0:1, 0:512], acc0[64:65, :])
            nc.vector.tensor_copy(dnx[0:1, 512:1024], acc1[64:65, :])
            nc.sync.dma_start(DN.ap(), dnx[:])
            for b in range(8):
                pb = scp.tile([128, 1024], F32, tag="sc")
                nc.tensor.matmul(pb[:, 0:512], U_sb[0:64, ts(b, 128)], ocb[0:64, :])
                nc.tensor.matmul(
                    pb[:, 512:1024], U_sb[64:128, ts(b, 128)], ocb[64:128, :]
                )
                st = stg.tile([128, 1024], BF16, tag="stb")
                if b % 2 == 0:
                    nc.scalar.copy(st[:, 0:512], pb[:, 0:512])
                    nc.vector.tensor_copy(st[:, 512:1024], pb[:, 512:1024])
                else:
                    nc.vector.tensor_copy(st[:, 0:512], pb[:, 0:512])
                    nc.scalar.copy(st[:, 512:1024], pb[:, 512:1024])
                (nc.sync if b % 2 == 0 else nc.scalar).dma_start(
                    PTB.ap()[ts(b, 128), :], st[:]
                )

    nc.compile()
    return nc


def _get_nc():
    if "nc" not in _CACHE:
        _CACHE["nc"] = _build_nc()
    return _CACHE["nc"]


def make_in_maps(Q, K, V, Wq, bq, Wk, bk, Wv, bv, Wo, bo):
    bf = ml_dtypes.bfloat16
    e4 = ml_dtypes.float8_e4m3
    in_maps = []
    for i in range(NCORES):
        c0 = i * CW
        h0, h1 = 2 * i, 2 * i + 1

        G_pack = np.zeros((CW, 64), np.float32)
        gq = np.zeros((CW, 1), np.float32)
        U_pack = np.zeros((CW, DK), np.float32)
        for hh, h in enumerate((h0, h1)):
            G_pack[hh * 64 : (hh + 1) * 64, :] = (Wq[h].T @ Wk[h]) / 8.0
            gq[hh * 64 : (hh + 1) * 64, 0] = (Wk[h].T @ bq[h]) / 8.0
            # U_pack[h*64+e, :] = (Wo_slice @ Wv[h])[:, e]
            U_pack[hh * 64 : (hh + 1) * 64, :] = (
                Wo[:, h * DH : (h + 1) * DH] @ Wv[h]
            ).T

        # VP layout per chunk-pair p: 2 k-tiles x 160 cols, each k-tile =
        # [h0 V 64 | ones | pad 15 | h1 V 64 | ones | pad 15]
        VP = np.zeros((CW, (NCH // 2) * VPW), np.float32)
        for p in range(NCH // 2):
            for i_t in range(2):
                j = 2 * p + i_t
                rows = slice(j * 128, (j + 1) * 128)
                for hh in range(2):
                    cbase = p * VPW + i_t * 160 + hh * 80
                    VP[:, cbase : cbase + 64] = V[rows, c0 + hh * 64 : c0 + (hh + 1) * 64]
                    VP[:, cbase + 64] = 1.0

        in_maps.append(
            {
                "QTd": np.ascontiguousarray(Q[:, c0 : c0 + CW].T).astype(bf),
                "KTd": np.ascontiguousarray(K[:, c0 : c0 + CW].T).astype(bf),
                "VPd": VP.astype(e4),
                "Gd": G_pack.astype(bf),
                "Ud": U_pack.astype(bf),
                "Gq": gq.astype(np.float32),
            }
        )
    return in_maps


def kernel(Q, K, V, Wq, bq, Wk, bk, Wv, bv, Wo, bo, _spmd_kwargs=None):
    Q, K, V = (np.asarray(x, np.float32) for x in (Q, K, V))
    Wq, bq, Wk, bk, Wv, bv = (
        np.asarray(x, np.float32) for x in (Wq, bq, Wk, bk, Wv, bv)
    )
    Wo, bo = np.asarray(Wo, np.float32), np.asarray(bo, np.float32)

    nc = _get_nc()
    in_maps = make_in_maps(Q, K, V, Wq, bq, Wk, bk, Wv, bv, Wo, bo)
    res = run_bass_kernel_spmd(
        nc, in_maps, core_ids=list(range(NCORES)), **(_spmd_kwargs or {})
    )

    acc = np.zeros((DK, S), np.float64)
    for i in range(NCORES):
        r = res.results[i]
        # PT cols 1536:2048 are never written on-device (PTB replaces them)
        acc[:, 0:1536] += r["PT"].astype(np.float64)[:, 0:1536]
        dn = r["DN"].astype(np.float64)[0]
        ptb = r["PTB"].astype(np.float64)
        acc[:, 1536:2048] += ptb[:, 0:512] / dn[0:512][None, :]
        acc[:, 1536:2048] += ptb[:, 512:1024] / dn[512:1024][None, :]
    bo_eff = bo + Wo @ bv.reshape(-1)
    out = (acc.T + bo_eff).astype(np.float32)
    if _spmd_kwargs:
        return out, res
    return out
